# revision 11
# baseline (speedup 1.0000x reference)
"""BasisLSTMCell Trainium2 kernel (8 NeuronCores, SPMD).

Sharding: 2-way data-parallel over batch x 4-way tensor-parallel over the
units dim.  Core c = g*4 + j handles batch rows [g*1024,(g+1)*1024) and
unit columns [j*256,(j+1)*256) of all four gates.

Math: z[b,n] = sum_{k,u} h[b,u] c[b,k] V[u,k,n]
            + sum_{k,e} x[b,e] c[b,k] W[e,k,n] + bias[n]
The contraction is laid out as the 64 recurrent (u,k) tiles in
expanding-square order, then the x-part r = k*96+e packed into 6 dense
tiles (xc = x*c_prob premultiplied on host), then (only when bias is
nonzero) one bias tile (row 0 = bias, against an all-ones stationary).
On-device, hcT[r,b] = hT[u,b]*cbT[k,b] tiles are built on VectorE from
host-pretransposed bf16 inputs; TensorE accumulates fp32 into PSUM;
ScalarE+VectorE run the LSTM pointwise.

Startup: the framework preamble blocks every engine until ~7.2us and a
DMA completion semaphore runs ~3us behind its descriptor, so the first
weight tile is split into four [128,256] chunks on the otherwise-empty
Vector queue (first descriptor in that queue -> earliest possible sem)
and tile 0's matmuls are emitted chunk-major at N=256 so the stream
starts as soon as chunk 0 lands.  Tiles 1-2 are halved nh-major on the
GpSimd queue; tiles 3-4 ride the Sync/Scalar queues between the
stream-critical source tiles.  A shortened HAM warmup (10 N=512
matmuls on garbage) spans the preamble->data window so the PE clock is
at 8/8 when real work arrives.

Each batch group's 8-bank accumulation is split into waves: wave A
(3 of 4 batch tiles) finishes with the main stream and its pointwise
epilogue overlaps the lagged tail of wave B (the 4th batch tile), whose
two PSUM banks are themselves staggered (nh=0 lands LAGB tiles early)
so the final sigmoid can start before the very last matmul.  The very
last matmul is split g-half-first and the final h store leaves on two
queues.
"""

import sys

for _p in ("/opt/trn_rl_repo", "/root/.axon_site/_ro/trn_rl_repo"):
    if _p not in sys.path:
        sys.path.insert(0, _p)

import numpy as np
from ml_dtypes import bfloat16

import concourse.bass as bass
import concourse.mybir as mybir
import concourse.tile as tile
from concourse.bass_utils import run_bass_kernel_spmd

B, E, K, U = 2048, 96, 8, 1024
G, J = 2, 4              # batch groups x column groups (G*J = 8 cores)
BC = B // G              # 1024 batch rows per core
NC = 4 * U // J          # 1024 z-columns per core
LAG = 13                 # wave-B (nh=0) lag in contraction tiles
LAGB = 3                 # extra lag for wave-B nh=1 behind nh=0
BF = mybir.dt.bfloat16
F32 = mybir.dt.float32
AF = mybir.ActivationFunctionType

# Recurrent contraction tiles are (u-block, k) pairs visited in expanding-
# square order: after the first m^2 tiles only 2m distinct source tiles
# (hT[u], cbT[k]) have been touched, so the early matmul stream never
# outruns the source DMAs.  The weight rows are permuted to match on the
# host.
ORDER = []
for _m in range(8):
    for _u in range(_m):
        ORDER.append((_u, _m))
    for _k in range(_m + 1):
        ORDER.append((_m, _k))

_cache = {}


def _split_excess_waits(nc, max_waits=1):
    """Walrus CoreV3 codegen accepts at most one sync-wait command per
    instruction; Tile's final drain can carry more.  Move the excess onto
    preceding same-engine NoOps (the engine executes in order, so the
    chain is semantically identical)."""
    for f in nc.m.functions:
        for bb in f.blocks:
            insts = list(bb.instructions)
            changed = False
            new = []
            for inst in insts:
                si = inst.sync_info
                if si is not None and len(si.on_wait) > max_waits:
                    waits = list(si.on_wait)
                    extra, keep = waits[:-max_waits], waits[-max_waits:]
                    for j in range(0, len(extra), max_waits):
                        new.append(mybir.InstNoOp(
                            name=f"{inst.name}-wsplit{j}",
                            engine=inst.engine,
                            sync_info=mybir.SyncInfo(
                                on_wait=extra[j:j + max_waits], on_update=[]),
                            bass_nofuse=True,
                        ))
                    inst.sync_info = mybir.SyncInfo(
                        on_wait=keep, on_update=si.on_update)
                    changed = True
                new.append(inst)
            if changed:
                while len(bb.instructions):
                    bb.instructions.pop()
                for i in new:
                    bb.instructions.append(i)


def _build(with_bias):
    ntt = 71 if with_bias else 70    # 64 recurrent + 6 x (+1 bias)
    nc = bass.Bass("TRN2", target_bir_lowering=False, debug=False,
                   num_devices=G * J)
    hT_d = nc.dram_tensor("hT", [8, 128, BC], BF, kind="ExternalInput").ap()
    xc_d = nc.dram_tensor("xc", [6, 128, BC], BF, kind="ExternalInput").ap()
    cbT_d = nc.dram_tensor("cbT", [8, 128, BC], BF, kind="ExternalInput").ap()
    Vw_d = nc.dram_tensor("Vw", [ntt, 128, NC], BF,
                          kind="ExternalInput").ap()
    cold_d = nc.dram_tensor("c_tm1", [BC, 256], F32,
                            kind="ExternalInput").ap()
    h_out_d = nc.dram_tensor("h_out", [BC, 256], F32,
                             kind="ExternalOutput").ap()
    c_out_d = nc.dram_tensor("c_out", [BC, 256], F32,
                             kind="ExternalOutput").ap()

    with tile.TileContext(nc) as tc:
        with tc.tile_pool(name="src", bufs=1) as srcp, \
             tc.tile_pool(name="w", bufs=LAG + LAGB + 12) as wp, \
             tc.tile_pool(name="hc", bufs=LAG + LAGB + 9) as hcp, \
             tc.tile_pool(name="pw", bufs=4) as pwp, \
             tc.tile_pool(name="io", bufs=3) as iop, \
             tc.tile_pool(name="psum", bufs=1, space="PSUM") as psp:

            # Early traffic is queue-bandwidth bound (gpsimd ~110-150,
            # sync ~50-70, scalar ~45-60 KB/us for the first ~20us) AND
            # ring-depth bound (~8 descriptors in flight per queue; a
            # 9th can't even start until the 1st finishes), so
            # everything before ~30us is halved to its mg0-critical part
            # and ordered by true need time (weight tile t consumed at
            # ~T0+1.72us*t; source hT[m]/cb[m] first read at tile ~m^2):
            #   gpsimd: w0h..w3h w4 | in-loop tiles | mg1 tiles
            #   sync:   hT0a hT1a w5h hT2a w9h hT3a..hT7a
            #           | hT0b..hT7b co0..co7
            #   scalar: cb0a..cb7a | xc* cb*b
            SYNC_W = (5, 9)
            w_pre = {}
            for t in (0, 1, 2, 3, 4) + SYNC_W:
                w_pre[t] = wp.tile([128, NC], BF, tag="w", name=f"wpre{t}")

            hT_t = [None] * 8
            cb_t = [None] * 8
            for i in range(8):
                hT_t[i] = srcp.tile([128, BC], BF, tag=f"hT{i}",
                                    name=f"hT{i}")
                cb_t[i] = srcp.tile([128, BC], BF, tag=f"cb{i}",
                                    name=f"cb{i}")

            def half(eng, dst, src, hf):
                sl = slice(hf * 512, hf * 512 + 512)
                eng.dma_start(out=dst[:, sl], in_=src[:, sl])

            # interleaved, need-ordered early schedule
            for t in range(4):
                half(nc.gpsimd, w_pre[t], Vw_d[t], 0)
                half(nc.gpsimd, w_pre[t], Vw_d[t], 1)
            nc.gpsimd.dma_start(out=w_pre[4][:, :], in_=Vw_d[4])
            half(nc.sync, hT_t[0], hT_d[0], 0)
            half(nc.scalar, cb_t[0], cbT_d[0], 0)
            half(nc.sync, hT_t[1], hT_d[1], 0)
            half(nc.scalar, cb_t[1], cbT_d[1], 0)
            half(nc.sync, w_pre[5], Vw_d[5], 0)
            half(nc.sync, w_pre[5], Vw_d[5], 1)
            half(nc.scalar, cb_t[2], cbT_d[2], 0)
            half(nc.sync, hT_t[2], hT_d[2], 0)
            half(nc.scalar, cb_t[3], cbT_d[3], 0)
            half(nc.sync, w_pre[9], Vw_d[9], 0)
            half(nc.sync, w_pre[9], Vw_d[9], 1)
            for i in range(4, 8):
                half(nc.scalar, cb_t[i], cbT_d[i], 0)
            for i in range(3, 8):
                half(nc.sync, hT_t[i], hT_d[i], 0)
            # mg1 halves + epilogue srcs, needed from ~130us onward
            for i in range(8):
                half(nc.sync, hT_t[i], hT_d[i], 1)
            xc_t = []
            for xt in range(6):
                t_ = srcp.tile([128, BC], BF, tag=f"xc{xt}", name=f"xc{xt}")
                nc.scalar.dma_start(out=t_[:, :], in_=xc_d[xt])
                xc_t.append(t_)
            co_t = [None] * 8
            for q in range(8):       # c_tm1 [128,256] per (mg, mloc)
                t_ = srcp.tile([128, 256], F32, tag=f"co{q}", name=f"co{q}")
                nc.sync.dma_start(out=t_[:, :],
                                  in_=cold_d[q * 128:(q + 1) * 128, :])
                co_t[q] = t_
            for i in range(8):
                half(nc.scalar, cb_t[i], cbT_d[i], 1)

            warm_src = nc.alloc_sbuf_tensor("warm_src", [128, 512], BF).ap()
            warm = psp.tile([128, 512], F32, tag="q0", name="warm")
            for _ in range(13):
                nc.tensor.matmul(warm[:, :], warm_src[:, 0:128],
                                 warm_src[:, 0:512], start=True, stop=True)

            if with_bias:
                ones_t = srcp.tile([128, 512], BF, tag="ones", name="ones")
                nc.vector.memset(ones_t[:, :], 1.0)

            def pw_head(bank, mloc, mg):
                # phase 1: everything that only needs p0 (i|f) + c_tm1
                p0 = bank[mloc]
                co = co_t[mg * 4 + mloc]
                if_s = pwp.tile([128, 512], F32, tag="if", name="ift")
                nc.scalar.activation(if_s[:, :], p0[:, :], AF.Sigmoid)
                t2 = pwp.tile([128, 256], F32, tag="t2", name="t2t")
                nc.vector.tensor_mul(t2[:, :], if_s[:, 256:512], co[:, :])
                return if_s, t2

            def pw_tail(bank, mloc, mg, if_s, t2, final=False):
                # phase 2: needs p1 (g|o); c_out is DMA'd before the h
                # chain so the last store is h only.  The kernel-final
                # variant ships the h store as two halves on two queues.
                row0 = mg * 512 + mloc * 128
                p1 = bank[4 + mloc]
                g_s = pwp.tile([128, 256], F32, tag="g", name="gt")
                nc.scalar.activation(g_s[:, :], p1[:, 0:256], AF.Tanh)
                t1 = pwp.tile([128, 256], F32, tag="t1", name="t1t")
                nc.vector.tensor_mul(t1[:, :], if_s[:, 0:256], g_s[:, :])
                c_n = iop.tile([128, 256], F32, tag="cn", name="cnt")
                nc.vector.tensor_add(c_n[:, :], t1[:, :], t2[:, :])
                nc.sync.dma_start(out=c_out_d[row0:row0 + 128, :],
                                  in_=c_n[:, :])
                o_s = pwp.tile([128, 256], F32, tag="o", name="ot")
                nc.scalar.activation(o_s[:, :], p1[:, 256:512], AF.Sigmoid)
                th = pwp.tile([128, 256], F32, tag="th", name="tht")
                nc.scalar.activation(th[:, :], c_n[:, :], AF.Tanh)
                h_n = iop.tile([128, 256], F32, tag="hn", name="hnt")
                nc.vector.tensor_mul(h_n[:, :], o_s[:, :], th[:, :])
                if final:
                    nc.sync.dma_start(out=h_out_d[row0:row0 + 128, 0:128],
                                      in_=h_n[:, 0:128])
                    nc.scalar.dma_start(out=h_out_d[row0:row0 + 128, 128:256],
                                        in_=h_n[:, 128:256])
                else:
                    nc.sync.dma_start(out=h_out_d[row0:row0 + 128, :],
                                      in_=h_n[:, :])

            def pointwise(bank, mlocs, mg):
                if len(mlocs) != 3:
                    for mloc in mlocs:
                        if_s, t2 = pw_head(bank, mloc, mg)
                        pw_tail(bank, mloc, mg, if_s, t2)
                    return
                # wave-A fused variant: the three c_n live in one tile so
                # a single [128,768] tanh replaces three [128,256] ones,
                # shortening the scalar queue that gates wave B's chain.
                c_all = iop.tile([128, 768], F32, tag="cna", name="cnat")
                th_all = pwp.tile([128, 768], F32, tag="tha", name="that")
                os_ = {}
                for i, mloc in enumerate(mlocs):
                    row0 = mg * 512 + mloc * 128
                    p0, p1 = bank[mloc], bank[4 + mloc]
                    co = co_t[mg * 4 + mloc]
                    if_s = pwp.tile([128, 512], F32, tag="if", name="ift")
                    nc.scalar.activation(if_s[:, :], p0[:, :], AF.Sigmoid)
                    t2 = pwp.tile([128, 256], F32, tag="t2", name="t2t")
                    nc.vector.tensor_mul(t2[:, :], if_s[:, 256:512],
                                         co[:, :])
                    g_s = pwp.tile([128, 256], F32, tag="g", name="gt")
                    nc.scalar.activation(g_s[:, :], p1[:, 0:256], AF.Tanh)
                    t1 = pwp.tile([128, 256], F32, tag="t1", name="t1t")
                    nc.vector.tensor_mul(t1[:, :], if_s[:, 0:256],
                                         g_s[:, :])
                    cs = slice(i * 256, (i + 1) * 256)
                    nc.vector.tensor_add(c_all[:, cs], t1[:, :], t2[:, :])
                    nc.sync.dma_start(out=c_out_d[row0:row0 + 128, :],
                                      in_=c_all[:, cs])
                    o_s = pwp.tile([128, 256], F32, tag="o", name="ot")
                    nc.scalar.activation(o_s[:, :], p1[:, 256:512],
                                         AF.Sigmoid)
                    os_[i] = o_s
                nc.scalar.activation(th_all[:, :], c_all[:, :], AF.Tanh)
                for i, mloc in enumerate(mlocs):
                    row0 = mg * 512 + mloc * 128
                    cs = slice(i * 256, (i + 1) * 256)
                    h_n = iop.tile([128, 256], F32, tag="hn", name="hnt")
                    nc.vector.tensor_mul(h_n[:, :], os_[i][:, :],
                                         th_all[:, cs])
                    nc.sync.dma_start(out=h_out_d[row0:row0 + 128, :],
                                      in_=h_n[:, :])

            for mg in range(2):           # groups of 4 batch tiles (512 rows)
                bsl = slice(mg * 512, (mg + 1) * 512)
                bank = [psp.tile([128, 512], F32, tag=f"q{q}", name=f"q{q}")
                        for q in range(8)]
                w_tiles = {}
                hc_tiles = {}

                def mm(t, mlocs, nhs=(0, 1), bank=bank, w_tiles=w_tiles,
                       hc_tiles=hc_tiles):
                    hc, w_t = hc_tiles[t], w_tiles[t]
                    for mloc in mlocs:
                        for nh in nhs:
                            nc.tensor.matmul(
                                bank[nh * 4 + mloc][:, :],
                                hc[:, mloc * 128:(mloc + 1) * 128],
                                w_t[:, nh * 512:(nh + 1) * 512],
                                start=(t == 0), stop=(t == ntt - 1))

                def mm_chunked(t, csize, bank=bank, w_tiles=w_tiles,
                               hc_tiles=hc_tiles):
                    # chunk-major emission: the first matmuls need only
                    # the first csize weight columns, so the stream can
                    # start before the whole tile has landed.
                    # start=True clears has_written for the WHOLE bank,
                    # so only the first chunk touching a bank may carry
                    # it; later chunks land on cleared bits and the
                    # accumulate-mode write overwrites there anyway.
                    hc, w_t = hc_tiles[t], w_tiles[t]
                    for c0 in range(0, NC, csize):
                        nh, p0 = c0 // 512, c0 % 512
                        for mloc in range(4):
                            nc.tensor.matmul(
                                bank[nh * 4 + mloc][:, p0:p0 + csize],
                                hc[:, mloc * 128:(mloc + 1) * 128],
                                w_t[:, c0:c0 + csize],
                                start=(t == 0 and p0 == 0),
                                stop=(t == ntt - 1))

                # mg0 runs all 8 banks in lockstep (weight-DMA demand at
                # startup stays at one tile per 8 matmuls); mg1 runs the
                # lagged wave schedule so its wave-A pointwise overlaps
                # the matmul tail (its weights prefetch during mg0).
                lag0 = 0 if mg == 0 else LAG
                lag1 = 0 if mg == 0 else LAG + LAGB
                wb_state = [None]
                for s in range(ntt + lag1):
                    if s < ntt:
                        t = s
                        if mg == 0 and t in w_pre:
                            w_t = w_pre[t]     # prefetched pre-warmup
                        else:
                            w_t = wp.tile([128, NC], BF, tag="w", name="wt")
                            # gpsimd queue: don't serialize behind srcs
                            nc.gpsimd.dma_start(out=w_t[:, :], in_=Vw_d[t])
                        w_tiles[t] = w_t
                        if t < 64:
                            ub, kb = ORDER[t]
                            hc = hcp.tile([128, 512], BF, tag="hc",
                                          name="hct")
                            nc.vector.tensor_mul(
                                hc[:, :], hT_t[ub][:, bsl],
                                cb_t[kb][:, bsl])
                        elif t < 70:
                            hc = xc_t[t - 64][:, bsl]
                        else:
                            hc = ones_t
                        hc_tiles[t] = hc
                        if mg == 0 and t in (0, 1, 2, 3, 4):
                            mm_chunked(t, 512)         # nh-major halves
                        else:
                            mm(t, (0, 1, 2))           # wave A
                            if mg == 0:
                                mm(t, (3,))
                    if mg == 1:
                        if lag0 <= s < ntt + lag0:
                            mm(s - lag0, (3,), nhs=(0,))    # wave B nh=0
                            if s - lag0 == ntt - 1:
                                wb_state[0] = pw_head(bank, 3, mg)
                        if lag1 <= s:
                            tb = s - lag1                   # wave B nh=1
                            if tb == ntt - 1:
                                # g-half first so the final tanh starts
                                # one matmul earlier
                                hcb, w_t = hc_tiles[tb], w_tiles[tb]
                                for p0 in (0, 256):
                                    nc.tensor.matmul(
                                        bank[7][:, p0:p0 + 256],
                                        hcb[:, 384:512],
                                        w_t[:, 512 + p0:768 + p0],
                                        start=False, stop=True)
                            else:
                                mm(tb, (3,), nhs=(1,))
                            del hc_tiles[tb]
                            del w_tiles[tb]
                    if s == ntt - 1:
                        pointwise(bank, (0, 1, 2), mg)   # wave A epilogue
                if mg == 0:
                    pointwise(bank, (3,), mg)
                else:
                    pw_tail(bank, 3, mg, *wb_state[0], final=True)

    _split_excess_waits(nc)
    return nc


def _prep_in_maps(inputs, h_tm1, c_tm1, basis_kernel, basis_recurrent_kernel,
                  bias, with_bias):
    inputs = np.asarray(inputs, np.float32)
    x = inputs[:, :E]
    c_prob = inputs[:, E:]
    h_tm1 = np.asarray(h_tm1, np.float32)
    c_tm1 = np.asarray(c_tm1, np.float32)

    Vr = np.asarray(basis_recurrent_kernel, np.float32) \
        .transpose(1, 0, 2).reshape(K, 8, 128, 4 * U)
    # permute the 64 recurrent tiles into the device's ORDER
    Vr = np.concatenate([Vr[kb, ub] for ub, kb in ORDER], 0)
    Wx = np.asarray(basis_kernel, np.float32) \
        .transpose(1, 0, 2).reshape(K * E, 4 * U)
    parts = [Vr, Wx]
    if with_bias:
        Bt = np.zeros((128, 4 * U), np.float32)
        Bt[0] = np.asarray(bias, np.float32)
        parts.append(Bt)
    Vw_full = np.concatenate(parts, 0)           # [ntt*128, 4096]
    ntt = Vw_full.shape[0] // 128

    er = np.arange(K * E) % E                    # x-part row -> e
    kr = np.arange(K * E) // E                   # x-part row -> k

    in_maps = []
    for g in range(G):
        bsl = slice(g * BC, (g + 1) * BC)
        hT = np.ascontiguousarray(h_tm1[bsl].T).astype(bfloat16) \
            .reshape(8, 128, BC)
        # x-part moving operand premultiplied on the host (one fp32
        # multiply + single bf16 round, slightly better than the
        # device's bf16*bf16)
        xc = np.ascontiguousarray(x[bsl].T[er] * c_prob[bsl].T[kr]) \
            .astype(bfloat16).reshape(6, 128, BC)
        cbT = np.ascontiguousarray(
            np.broadcast_to(c_prob[bsl].T[:, None, :], (8, 128, BC))
        ).astype(bfloat16)
        for j in range(J):
            cols = np.concatenate(
                [np.arange(gt * U + j * 256, gt * U + (j + 1) * 256)
                 for gt in range(4)])
            Vw_c = np.ascontiguousarray(Vw_full[:, cols]).astype(bfloat16) \
                .reshape(ntt, 128, NC)
            co = np.ascontiguousarray(c_tm1[bsl, j * 256:(j + 1) * 256])
            in_maps.append({"hT": hT, "xc": xc, "cbT": cbT,
                            "Vw": Vw_c, "c_tm1": co})
    return in_maps


def _run(in_maps, with_bias, trace=False, **kw):
    key = f"nc{int(with_bias)}"
    if key not in _cache:
        _cache[key] = _build(with_bias)
    try:
        return run_bass_kernel_spmd(_cache[key], in_maps,
                                    list(range(G * J)), trace=trace, **kw)
    except Exception:
        # transient device-state failures have been observed to clear on
        # the next execution; retry once without tracing
        return run_bass_kernel_spmd(_cache[key], in_maps,
                                    list(range(G * J)), trace=False)


def kernel(inputs, h_tm1, c_tm1, basis_kernel, basis_recurrent_kernel, bias,
           _trace=False, **_kw):
    with_bias = bool(np.any(np.asarray(bias)))
    in_maps = _prep_in_maps(inputs, h_tm1, c_tm1, basis_kernel,
                            basis_recurrent_kernel, bias, with_bias)
    res = _run(in_maps, with_bias, trace=_trace, **_kw)
    h = np.empty((B, U), np.float32)
    c = np.empty((B, U), np.float32)
    for g in range(G):
        for j in range(J):
            r = res.results[g * J + j]
            h[g * BC:(g + 1) * BC, j * 256:(j + 1) * 256] = r["h_out"]
            c[g * BC:(g + 1) * BC, j * 256:(j + 1) * 256] = r["c_out"]
    kernel.last_results = res
    return (h, c)


# revision 14
# speedup vs baseline: 1.0399x; 1.0399x over previous
"""BasisLSTMCell Trainium2 kernel (8 NeuronCores, SPMD).

Sharding: 2-way data-parallel over batch x 4-way tensor-parallel over the
units dim.  Core c = g*4 + j handles batch rows [g*1024,(g+1)*1024) and
unit columns [j*256,(j+1)*256) of all four gates.

Math: z[b,n] = sum_{k,u} h[b,u] c[b,k] V[u,k,n]
            + sum_{k,e} x[b,e] c[b,k] W[e,k,n] + bias[n]
The contraction is laid out as the 64 recurrent (u,k) tiles in
expanding-square order, then the x-part r = k*96+e packed into 6 dense
tiles (xc = x*c_prob premultiplied on host), then (only when bias is
nonzero) one bias tile (row 0 = bias, against an all-ones stationary).
On-device, hcT[r,b] = hT[u,b]*cbT[k,b] tiles are built on VectorE from
host-pretransposed bf16 inputs; TensorE accumulates fp32 into PSUM;
ScalarE+VectorE run the LSTM pointwise.

Startup: the framework preamble blocks every engine until ~7.2us and a
DMA completion semaphore runs ~3us behind its descriptor, so the first
weight tile is split into four [128,256] chunks on the otherwise-empty
Vector queue (first descriptor in that queue -> earliest possible sem)
and tile 0's matmuls are emitted chunk-major at N=256 so the stream
starts as soon as chunk 0 lands.  Tiles 1-2 are halved nh-major on the
GpSimd queue; tiles 3-4 ride the Sync/Scalar queues between the
stream-critical source tiles.  A shortened HAM warmup (10 N=512
matmuls on garbage) spans the preamble->data window so the PE clock is
at 8/8 when real work arrives.

Each batch group's 8-bank accumulation is split into waves: wave A
(3 of 4 batch tiles) finishes with the main stream and its pointwise
epilogue overlaps the lagged tail of wave B (the 4th batch tile), whose
two PSUM banks are themselves staggered (nh=0 lands LAGB tiles early)
so the final sigmoid can start before the very last matmul.  The very
last matmul is split g-half-first and the final h store leaves on two
queues.
"""

import sys

for _p in ("/opt/trn_rl_repo", "/root/.axon_site/_ro/trn_rl_repo"):
    if _p not in sys.path:
        sys.path.insert(0, _p)

import numpy as np
from ml_dtypes import bfloat16

import concourse.bass as bass
import concourse.mybir as mybir
import concourse.tile as tile
from concourse.bass_utils import run_bass_kernel_spmd

B, E, K, U = 2048, 96, 8, 1024
G, J = 2, 4              # batch groups x column groups (G*J = 8 cores)
BC = B // G              # 1024 batch rows per core
NC = 4 * U // J          # 1024 z-columns per core
LAG = 13                 # wave-B (nh=0) lag in contraction tiles
LAGB = 3                 # extra lag for wave-B nh=1 behind nh=0
BF = mybir.dt.bfloat16
F32 = mybir.dt.float32
AF = mybir.ActivationFunctionType

# Recurrent contraction tiles are (u-block, k) pairs visited in expanding-
# square order: after the first m^2 tiles only 2m distinct source tiles
# (hT[u], cbT[k]) have been touched, so the early matmul stream never
# outruns the source DMAs.  The weight rows are permuted to match on the
# host.
ORDER = []
for _m in range(8):
    for _u in range(_m):
        ORDER.append((_u, _m))
    for _k in range(_m + 1):
        ORDER.append((_m, _k))

_cache = {}


def _split_excess_waits(nc, max_waits=1):
    """Walrus CoreV3 codegen accepts at most one sync-wait command per
    instruction; Tile's final drain can carry more.  Move the excess onto
    preceding same-engine NoOps (the engine executes in order, so the
    chain is semantically identical)."""
    for f in nc.m.functions:
        for bb in f.blocks:
            insts = list(bb.instructions)
            changed = False
            new = []
            for inst in insts:
                si = inst.sync_info
                if si is not None and len(si.on_wait) > max_waits:
                    waits = list(si.on_wait)
                    extra, keep = waits[:-max_waits], waits[-max_waits:]
                    for j in range(0, len(extra), max_waits):
                        new.append(mybir.InstNoOp(
                            name=f"{inst.name}-wsplit{j}",
                            engine=inst.engine,
                            sync_info=mybir.SyncInfo(
                                on_wait=extra[j:j + max_waits], on_update=[]),
                            bass_nofuse=True,
                        ))
                    inst.sync_info = mybir.SyncInfo(
                        on_wait=keep, on_update=si.on_update)
                    changed = True
                new.append(inst)
            if changed:
                while len(bb.instructions):
                    bb.instructions.pop()
                for i in new:
                    bb.instructions.append(i)


def _build(with_bias):
    ntt = 71 if with_bias else 70    # 64 recurrent + 6 x (+1 bias)
    nc = bass.Bass("TRN2", target_bir_lowering=False, debug=False,
                   num_devices=G * J)
    hT_d = nc.dram_tensor("hT", [8, 128, BC], BF, kind="ExternalInput").ap()
    xc_d = nc.dram_tensor("xc", [6, 128, BC], BF, kind="ExternalInput").ap()
    cbT_d = nc.dram_tensor("cbT", [8, 128, BC], BF, kind="ExternalInput").ap()
    Vw_d = nc.dram_tensor("Vw", [ntt, 128, NC], BF,
                          kind="ExternalInput").ap()
    cold_d = nc.dram_tensor("c_tm1", [BC, 256], F32,
                            kind="ExternalInput").ap()
    h_out_d = nc.dram_tensor("h_out", [BC, 256], F32,
                             kind="ExternalOutput").ap()
    c_out_d = nc.dram_tensor("c_out", [BC, 256], F32,
                             kind="ExternalOutput").ap()

    with tile.TileContext(nc) as tc:
        with tc.tile_pool(name="src", bufs=1) as srcp, \
             tc.tile_pool(name="w", bufs=LAG + LAGB + 12) as wp, \
             tc.tile_pool(name="hc", bufs=LAG + LAGB + 9) as hcp, \
             tc.tile_pool(name="pw", bufs=4) as pwp, \
             tc.tile_pool(name="io", bufs=3) as iop, \
             tc.tile_pool(name="psum", bufs=1, space="PSUM") as psp:

            # Early traffic is queue-bandwidth bound (gpsimd ~110-150,
            # sync ~50-70, scalar ~45-60 KB/us for the first ~20us) AND
            # ring-depth bound (~8 descriptors in flight per queue; a
            # 9th can't even start until the 1st finishes), so
            # everything before ~30us is halved to its mg0-critical part
            # and ordered by true need time (weight tile t consumed at
            # ~T0+1.72us*t; source hT[m]/cb[m] first read at tile ~m^2):
            #   gpsimd: w0h..w3h w4 | in-loop tiles | mg1 tiles
            #   sync:   hT0a..hT7a | hT0b..hT7b co0..co7
            #   scalar: cb0a..cb7a | xc* cb*b
            w_pre = {}
            for t in (0, 1, 2, 3, 4):
                w_pre[t] = wp.tile([128, NC], BF, tag="w", name=f"wpre{t}")

            hT_t = [None] * 8
            cb_t = [None] * 8
            for i in range(8):
                hT_t[i] = srcp.tile([128, BC], BF, tag=f"hT{i}",
                                    name=f"hT{i}")
                cb_t[i] = srcp.tile([128, BC], BF, tag=f"cb{i}",
                                    name=f"cb{i}")

            def half(eng, dst, src, hf):
                sl = slice(hf * 512, hf * 512 + 512)
                eng.dma_start(out=dst[:, sl], in_=src[:, sl])

            # interleaved, need-ordered early schedule
            for t in range(4):
                half(nc.gpsimd, w_pre[t], Vw_d[t], 0)
                half(nc.gpsimd, w_pre[t], Vw_d[t], 1)
            nc.gpsimd.dma_start(out=w_pre[4][:, :], in_=Vw_d[4])
            half(nc.sync, hT_t[0], hT_d[0], 0)
            half(nc.scalar, cb_t[0], cbT_d[0], 0)
            for i in range(1, 8):
                half(nc.sync, hT_t[i], hT_d[i], 0)
                half(nc.scalar, cb_t[i], cbT_d[i], 0)
            # mg1 halves + epilogue srcs, needed from ~130us onward
            for i in range(8):
                half(nc.sync, hT_t[i], hT_d[i], 1)
            xc_t = []
            for xt in range(6):
                t_ = srcp.tile([128, BC], BF, tag=f"xc{xt}", name=f"xc{xt}")
                nc.scalar.dma_start(out=t_[:, :], in_=xc_d[xt])
                xc_t.append(t_)
            co_t = [None] * 8
            for q in range(8):       # c_tm1 [128,256] per (mg, mloc)
                t_ = srcp.tile([128, 256], F32, tag=f"co{q}", name=f"co{q}")
                nc.sync.dma_start(out=t_[:, :],
                                  in_=cold_d[q * 128:(q + 1) * 128, :])
                co_t[q] = t_
            for i in range(8):
                half(nc.scalar, cb_t[i], cbT_d[i], 1)

            warm_src = nc.alloc_sbuf_tensor("warm_src", [128, 512], BF).ap()
            warm = psp.tile([128, 512], F32, tag="q0", name="warm")
            for _ in range(16):
                nc.tensor.matmul(warm[:, :], warm_src[:, 0:128],
                                 warm_src[:, 0:512], start=True, stop=True)

            if with_bias:
                ones_t = srcp.tile([128, 512], BF, tag="ones", name="ones")
                nc.vector.memset(ones_t[:, :], 1.0)

            def pw_head(bank, mloc, mg):
                # phase 1: everything that only needs p0 (i|f) + c_tm1
                p0 = bank[mloc]
                co = co_t[mg * 4 + mloc]
                if_s = pwp.tile([128, 512], F32, tag="if", name="ift")
                nc.scalar.activation(if_s[:, :], p0[:, :], AF.Sigmoid)
                t2 = pwp.tile([128, 256], F32, tag="t2", name="t2t")
                nc.vector.tensor_mul(t2[:, :], if_s[:, 256:512], co[:, :])
                return if_s, t2

            def pw_tail(bank, mloc, mg, if_s, t2, final=False):
                # phase 2: needs p1 (g|o); c_out is DMA'd before the h
                # chain so the last store is h only.  The kernel-final
                # variant ships the h store as two halves on two queues.
                row0 = mg * 512 + mloc * 128
                p1 = bank[4 + mloc]
                g_s = pwp.tile([128, 256], F32, tag="g", name="gt")
                nc.scalar.activation(g_s[:, :], p1[:, 0:256], AF.Tanh)
                t1 = pwp.tile([128, 256], F32, tag="t1", name="t1t")
                nc.vector.tensor_mul(t1[:, :], if_s[:, 0:256], g_s[:, :])
                c_n = iop.tile([128, 256], F32, tag="cn", name="cnt")
                nc.vector.tensor_add(c_n[:, :], t1[:, :], t2[:, :])
                nc.sync.dma_start(out=c_out_d[row0:row0 + 128, :],
                                  in_=c_n[:, :])
                o_s = pwp.tile([128, 256], F32, tag="o", name="ot")
                nc.scalar.activation(o_s[:, :], p1[:, 256:512], AF.Sigmoid)
                th = pwp.tile([128, 256], F32, tag="th", name="tht")
                nc.scalar.activation(th[:, :], c_n[:, :], AF.Tanh)
                h_n = iop.tile([128, 256], F32, tag="hn", name="hnt")
                nc.vector.tensor_mul(h_n[:, :], o_s[:, :], th[:, :])
                if final:
                    nc.sync.dma_start(out=h_out_d[row0:row0 + 128, 0:128],
                                      in_=h_n[:, 0:128])
                    nc.scalar.dma_start(out=h_out_d[row0:row0 + 128, 128:256],
                                        in_=h_n[:, 128:256])
                else:
                    nc.sync.dma_start(out=h_out_d[row0:row0 + 128, :],
                                      in_=h_n[:, :])

            def pointwise(bank, mlocs, mg):
                if len(mlocs) != 3:
                    for mloc in mlocs:
                        if_s, t2 = pw_head(bank, mloc, mg)
                        pw_tail(bank, mloc, mg, if_s, t2)
                    return
                # wave-A fused variant: the three c_n live in one tile so
                # a single [128,768] tanh replaces three [128,256] ones,
                # shortening the scalar queue that gates wave B's chain.
                c_all = iop.tile([128, 768], F32, tag="cna", name="cnat")
                th_all = pwp.tile([128, 768], F32, tag="tha", name="that")
                os_ = {}
                for i, mloc in enumerate(mlocs):
                    row0 = mg * 512 + mloc * 128
                    p0, p1 = bank[mloc], bank[4 + mloc]
                    co = co_t[mg * 4 + mloc]
                    if_s = pwp.tile([128, 512], F32, tag="if", name="ift")
                    nc.scalar.activation(if_s[:, :], p0[:, :], AF.Sigmoid)
                    t2 = pwp.tile([128, 256], F32, tag="t2", name="t2t")
                    nc.vector.tensor_mul(t2[:, :], if_s[:, 256:512],
                                         co[:, :])
                    g_s = pwp.tile([128, 256], F32, tag="g", name="gt")
                    nc.scalar.activation(g_s[:, :], p1[:, 0:256], AF.Tanh)
                    t1 = pwp.tile([128, 256], F32, tag="t1", name="t1t")
                    nc.vector.tensor_mul(t1[:, :], if_s[:, 0:256],
                                         g_s[:, :])
                    cs = slice(i * 256, (i + 1) * 256)
                    nc.vector.tensor_add(c_all[:, cs], t1[:, :], t2[:, :])
                    nc.sync.dma_start(out=c_out_d[row0:row0 + 128, :],
                                      in_=c_all[:, cs])
                    o_s = pwp.tile([128, 256], F32, tag="o", name="ot")
                    nc.scalar.activation(o_s[:, :], p1[:, 256:512],
                                         AF.Sigmoid)
                    os_[i] = o_s
                nc.scalar.activation(th_all[:, :], c_all[:, :], AF.Tanh)
                for i, mloc in enumerate(mlocs):
                    row0 = mg * 512 + mloc * 128
                    cs = slice(i * 256, (i + 1) * 256)
                    h_n = iop.tile([128, 256], F32, tag="hn", name="hnt")
                    nc.vector.tensor_mul(h_n[:, :], os_[i][:, :],
                                         th_all[:, cs])
                    nc.sync.dma_start(out=h_out_d[row0:row0 + 128, :],
                                      in_=h_n[:, :])

            for mg in range(2):           # groups of 4 batch tiles (512 rows)
                bsl = slice(mg * 512, (mg + 1) * 512)
                bank = [psp.tile([128, 512], F32, tag=f"q{q}", name=f"q{q}")
                        for q in range(8)]
                w_tiles = {}
                hc_tiles = {}

                def mm(t, mlocs, nhs=(0, 1), bank=bank, w_tiles=w_tiles,
                       hc_tiles=hc_tiles):
                    hc, w_t = hc_tiles[t], w_tiles[t]
                    for mloc in mlocs:
                        for nh in nhs:
                            nc.tensor.matmul(
                                bank[nh * 4 + mloc][:, :],
                                hc[:, mloc * 128:(mloc + 1) * 128],
                                w_t[:, nh * 512:(nh + 1) * 512],
                                start=(t == 0), stop=(t == ntt - 1))

                def mm_chunked(t, csize, bank=bank, w_tiles=w_tiles,
                               hc_tiles=hc_tiles):
                    # chunk-major emission: the first matmuls need only
                    # the first csize weight columns, so the stream can
                    # start before the whole tile has landed.
                    # start=True clears has_written for the WHOLE bank,
                    # so only the first chunk touching a bank may carry
                    # it; later chunks land on cleared bits and the
                    # accumulate-mode write overwrites there anyway.
                    hc, w_t = hc_tiles[t], w_tiles[t]
                    for c0 in range(0, NC, csize):
                        nh, p0 = c0 // 512, c0 % 512
                        for mloc in range(4):
                            nc.tensor.matmul(
                                bank[nh * 4 + mloc][:, p0:p0 + csize],
                                hc[:, mloc * 128:(mloc + 1) * 128],
                                w_t[:, c0:c0 + csize],
                                start=(t == 0 and p0 == 0),
                                stop=(t == ntt - 1))

                # mg0 runs all 8 banks in lockstep (weight-DMA demand at
                # startup stays at one tile per 8 matmuls); mg1 runs the
                # lagged wave schedule so its wave-A pointwise overlaps
                # the matmul tail (its weights prefetch during mg0).
                lag0 = 0 if mg == 0 else LAG
                lag1 = 0 if mg == 0 else LAG + LAGB
                wb_state = [None]
                for s in range(ntt + lag1):
                    if s < ntt:
                        t = s
                        if mg == 0 and t in w_pre:
                            w_t = w_pre[t]     # prefetched pre-warmup
                        else:
                            w_t = wp.tile([128, NC], BF, tag="w", name="wt")
                            # gpsimd queue: don't serialize behind srcs
                            nc.gpsimd.dma_start(out=w_t[:, :], in_=Vw_d[t])
                        w_tiles[t] = w_t
                        if t < 64:
                            ub, kb = ORDER[t]
                            hc = hcp.tile([128, 512], BF, tag="hc",
                                          name="hct")
                            nc.vector.tensor_mul(
                                hc[:, :], hT_t[ub][:, bsl],
                                cb_t[kb][:, bsl])
                        elif t < 70:
                            hc = xc_t[t - 64][:, bsl]
                        else:
                            hc = ones_t
                        hc_tiles[t] = hc
                        if mg == 0 and t in (0, 1, 2, 3, 4):
                            mm_chunked(t, 512)         # nh-major halves
                        else:
                            mm(t, (0, 1, 2))           # wave A
                            if mg == 0:
                                mm(t, (3,))
                    if mg == 1:
                        if lag0 <= s < ntt + lag0:
                            mm(s - lag0, (3,), nhs=(0,))    # wave B nh=0
                            if s - lag0 == ntt - 1:
                                wb_state[0] = pw_head(bank, 3, mg)
                        if lag1 <= s:
                            tb = s - lag1                   # wave B nh=1
                            if tb == ntt - 1:
                                # g-half first so the final tanh starts
                                # one matmul earlier
                                hcb, w_t = hc_tiles[tb], w_tiles[tb]
                                for p0 in (0, 256):
                                    nc.tensor.matmul(
                                        bank[7][:, p0:p0 + 256],
                                        hcb[:, 384:512],
                                        w_t[:, 512 + p0:768 + p0],
                                        start=False, stop=True)
                            else:
                                mm(tb, (3,), nhs=(1,))
                            del hc_tiles[tb]
                            del w_tiles[tb]
                    if s == ntt - 1:
                        pointwise(bank, (0, 1, 2), mg)   # wave A epilogue
                if mg == 0:
                    pointwise(bank, (3,), mg)
                else:
                    pw_tail(bank, 3, mg, *wb_state[0], final=True)

    _split_excess_waits(nc)
    return nc


def _prep_in_maps(inputs, h_tm1, c_tm1, basis_kernel, basis_recurrent_kernel,
                  bias, with_bias):
    inputs = np.asarray(inputs, np.float32)
    x = inputs[:, :E]
    c_prob = inputs[:, E:]
    h_tm1 = np.asarray(h_tm1, np.float32)
    c_tm1 = np.asarray(c_tm1, np.float32)

    Vr = np.asarray(basis_recurrent_kernel, np.float32) \
        .transpose(1, 0, 2).reshape(K, 8, 128, 4 * U)
    # permute the 64 recurrent tiles into the device's ORDER
    Vr = np.concatenate([Vr[kb, ub] for ub, kb in ORDER], 0)
    Wx = np.asarray(basis_kernel, np.float32) \
        .transpose(1, 0, 2).reshape(K * E, 4 * U)
    parts = [Vr, Wx]
    if with_bias:
        Bt = np.zeros((128, 4 * U), np.float32)
        Bt[0] = np.asarray(bias, np.float32)
        parts.append(Bt)
    Vw_full = np.concatenate(parts, 0)           # [ntt*128, 4096]
    ntt = Vw_full.shape[0] // 128

    er = np.arange(K * E) % E                    # x-part row -> e
    kr = np.arange(K * E) // E                   # x-part row -> k

    in_maps = []
    for g in range(G):
        bsl = slice(g * BC, (g + 1) * BC)
        hT = np.ascontiguousarray(h_tm1[bsl].T).astype(bfloat16) \
            .reshape(8, 128, BC)
        # x-part moving operand premultiplied on the host (one fp32
        # multiply + single bf16 round, slightly better than the
        # device's bf16*bf16)
        xc = np.ascontiguousarray(x[bsl].T[er] * c_prob[bsl].T[kr]) \
            .astype(bfloat16).reshape(6, 128, BC)
        cbT = np.ascontiguousarray(
            np.broadcast_to(c_prob[bsl].T[:, None, :], (8, 128, BC))
        ).astype(bfloat16)
        for j in range(J):
            cols = np.concatenate(
                [np.arange(gt * U + j * 256, gt * U + (j + 1) * 256)
                 for gt in range(4)])
            Vw_c = np.ascontiguousarray(Vw_full[:, cols]).astype(bfloat16) \
                .reshape(ntt, 128, NC)
            co = np.ascontiguousarray(c_tm1[bsl, j * 256:(j + 1) * 256])
            in_maps.append({"hT": hT, "xc": xc, "cbT": cbT,
                            "Vw": Vw_c, "c_tm1": co})
    return in_maps


def _run(in_maps, with_bias, trace=False, **kw):
    key = f"nc{int(with_bias)}"
    if key not in _cache:
        _cache[key] = _build(with_bias)
    try:
        return run_bass_kernel_spmd(_cache[key], in_maps,
                                    list(range(G * J)), trace=trace, **kw)
    except Exception:
        # transient device-state failures have been observed to clear on
        # the next execution; retry once without tracing
        return run_bass_kernel_spmd(_cache[key], in_maps,
                                    list(range(G * J)), trace=False)


def kernel(inputs, h_tm1, c_tm1, basis_kernel, basis_recurrent_kernel, bias,
           _trace=False, **_kw):
    with_bias = bool(np.any(np.asarray(bias)))
    in_maps = _prep_in_maps(inputs, h_tm1, c_tm1, basis_kernel,
                            basis_recurrent_kernel, bias, with_bias)
    res = _run(in_maps, with_bias, trace=_trace, **_kw)
    h = np.empty((B, U), np.float32)
    c = np.empty((B, U), np.float32)
    for g in range(G):
        for j in range(J):
            r = res.results[g * J + j]
            h[g * BC:(g + 1) * BC, j * 256:(j + 1) * 256] = r["h_out"]
            c[g * BC:(g + 1) * BC, j * 256:(j + 1) * 256] = r["c_out"]
    kernel.last_results = res
    return (h, c)


# revision 16
# speedup vs baseline: 1.0413x; 1.0013x over previous
"""BasisLSTMCell Trainium2 kernel (8 NeuronCores, SPMD).

Sharding: 2-way data-parallel over batch x 4-way tensor-parallel over the
units dim.  Core c = g*4 + j handles batch rows [g*1024,(g+1)*1024) and
unit columns [j*256,(j+1)*256) of all four gates.

Math: z[b,n] = sum_{k,u} h[b,u] c[b,k] V[u,k,n]
            + sum_{k,e} x[b,e] c[b,k] W[e,k,n] + bias[n]
The contraction is laid out as the 64 recurrent (u,k) tiles in
expanding-square order, then the x-part r = k*96+e packed into 6 dense
tiles (xc = x*c_prob premultiplied on host), then (only when bias is
nonzero) one bias tile (row 0 = bias, against an all-ones stationary).
On-device, hcT[r,b] = hT[u,b]*cbT[k,b] tiles are built on VectorE from
host-pretransposed bf16 inputs; TensorE accumulates fp32 into PSUM;
ScalarE+VectorE run the LSTM pointwise.

Startup: the framework preamble blocks every engine until ~7.2us and a
DMA completion semaphore runs ~3us behind its descriptor, so the first
weight tile is split into four [128,256] chunks on the otherwise-empty
Vector queue (first descriptor in that queue -> earliest possible sem)
and tile 0's matmuls are emitted chunk-major at N=256 so the stream
starts as soon as chunk 0 lands.  Tiles 1-2 are halved nh-major on the
GpSimd queue; tiles 3-4 ride the Sync/Scalar queues between the
stream-critical source tiles.  A shortened HAM warmup (10 N=512
matmuls on garbage) spans the preamble->data window so the PE clock is
at 8/8 when real work arrives.

Each batch group's 8-bank accumulation is split into waves: wave A
(3 of 4 batch tiles) finishes with the main stream and its pointwise
epilogue overlaps the lagged tail of wave B (the 4th batch tile), whose
two PSUM banks are themselves staggered (nh=0 lands LAGB tiles early)
so the final sigmoid can start before the very last matmul.  The very
last matmul is split g-half-first and the final h store leaves on two
queues.
"""

import sys

for _p in ("/opt/trn_rl_repo", "/root/.axon_site/_ro/trn_rl_repo"):
    if _p not in sys.path:
        sys.path.insert(0, _p)

import numpy as np
from ml_dtypes import bfloat16

import concourse.bass as bass
import concourse.mybir as mybir
import concourse.tile as tile
from concourse.bass_utils import run_bass_kernel_spmd

B, E, K, U = 2048, 96, 8, 1024
G, J = 2, 4              # batch groups x column groups (G*J = 8 cores)
BC = B // G              # 1024 batch rows per core
NC = 4 * U // J          # 1024 z-columns per core
LAG = 13                 # wave-B (nh=0) lag in contraction tiles
LAGB = 3                 # extra lag for wave-B nh=1 behind nh=0
BF = mybir.dt.bfloat16
F32 = mybir.dt.float32
AF = mybir.ActivationFunctionType

# Recurrent contraction tiles are (u-block, k) pairs visited in expanding-
# square order: after the first m^2 tiles only 2m distinct source tiles
# (hT[u], cbT[k]) have been touched, so the early matmul stream never
# outruns the source DMAs.  The weight rows are permuted to match on the
# host.
ORDER = []
for _m in range(8):
    for _u in range(_m):
        ORDER.append((_u, _m))
    for _k in range(_m + 1):
        ORDER.append((_m, _k))

_cache = {}


def _split_excess_waits(nc, max_waits=1):
    """Walrus CoreV3 codegen accepts at most one sync-wait command per
    instruction; Tile's final drain can carry more.  Move the excess onto
    preceding same-engine NoOps (the engine executes in order, so the
    chain is semantically identical)."""
    for f in nc.m.functions:
        for bb in f.blocks:
            insts = list(bb.instructions)
            changed = False
            new = []
            for inst in insts:
                si = inst.sync_info
                if si is not None and len(si.on_wait) > max_waits:
                    waits = list(si.on_wait)
                    extra, keep = waits[:-max_waits], waits[-max_waits:]
                    for j in range(0, len(extra), max_waits):
                        new.append(mybir.InstNoOp(
                            name=f"{inst.name}-wsplit{j}",
                            engine=inst.engine,
                            sync_info=mybir.SyncInfo(
                                on_wait=extra[j:j + max_waits], on_update=[]),
                            bass_nofuse=True,
                        ))
                    inst.sync_info = mybir.SyncInfo(
                        on_wait=keep, on_update=si.on_update)
                    changed = True
                new.append(inst)
            if changed:
                while len(bb.instructions):
                    bb.instructions.pop()
                for i in new:
                    bb.instructions.append(i)


def _build(with_bias):
    ntt = 71 if with_bias else 70    # 64 recurrent + 6 x (+1 bias)
    nc = bass.Bass("TRN2", target_bir_lowering=False, debug=False,
                   num_devices=G * J)
    hT_d = nc.dram_tensor("hT", [8, 128, BC], BF, kind="ExternalInput").ap()
    xc_d = nc.dram_tensor("xc", [6, 128, BC], BF, kind="ExternalInput").ap()
    cbT_d = nc.dram_tensor("cbT", [8, 128, BC], BF, kind="ExternalInput").ap()
    Vw_d = nc.dram_tensor("Vw", [ntt, 128, NC], BF,
                          kind="ExternalInput").ap()
    cold_d = nc.dram_tensor("c_tm1", [BC, 256], F32,
                            kind="ExternalInput").ap()
    h_out_d = nc.dram_tensor("h_out", [BC, 256], F32,
                             kind="ExternalOutput").ap()
    c_out_d = nc.dram_tensor("c_out", [BC, 256], F32,
                             kind="ExternalOutput").ap()

    with tile.TileContext(nc) as tc:
        with tc.tile_pool(name="src", bufs=1) as srcp, \
             tc.tile_pool(name="w", bufs=LAG + LAGB + 12) as wp, \
             tc.tile_pool(name="hc", bufs=LAG + LAGB + 9) as hcp, \
             tc.tile_pool(name="pw", bufs=4) as pwp, \
             tc.tile_pool(name="io", bufs=3) as iop, \
             tc.tile_pool(name="psum", bufs=1, space="PSUM") as psp:

            # Early traffic is queue-bandwidth bound (gpsimd ~110-150,
            # sync ~50-70, scalar ~45-60 KB/us for the first ~20us) AND
            # ring-depth bound (~8 descriptors in flight per queue; a
            # 9th can't even start until the 1st finishes), so
            # everything before ~30us is halved to its mg0-critical part
            # and ordered by true need time (weight tile t consumed at
            # ~T0+1.72us*t; source hT[m]/cb[m] first read at tile ~m^2):
            #   gpsimd: w0h..w3h w4 | in-loop tiles | mg1 tiles
            #   sync:   hT0a..hT7a | hT0b..hT7b co0..co7
            #   scalar: cb0a..cb7a | xc* cb*b
            w_pre = {}
            for t in (0, 1, 2, 3, 4):
                w_pre[t] = wp.tile([128, NC], BF, tag="w", name=f"wpre{t}")

            hT_t = [None] * 8
            cb_t = [None] * 8
            for i in range(8):
                hT_t[i] = srcp.tile([128, BC], BF, tag=f"hT{i}",
                                    name=f"hT{i}")
                cb_t[i] = srcp.tile([128, BC], BF, tag=f"cb{i}",
                                    name=f"cb{i}")

            def half(eng, dst, src, hf):
                sl = slice(hf * 512, hf * 512 + 512)
                eng.dma_start(out=dst[:, sl], in_=src[:, sl])

            # interleaved, need-ordered early schedule.  w0's halves ride
            # two different queues so the stream-start gate pays one
            # straggler-engine delay, not two in series; w1/w2 are
            # 4-chunked so partial arrivals feed the chunk-major matmuls.
            half(nc.gpsimd, w_pre[0], Vw_d[0], 0)
            half(nc.sync, w_pre[0], Vw_d[0], 1)
            for t in (1, 2):
                for ci in range(4):
                    nc.gpsimd.dma_start(
                        out=w_pre[t][:, ci * 256:(ci + 1) * 256],
                        in_=Vw_d[t][:, ci * 256:(ci + 1) * 256])
            half(nc.gpsimd, w_pre[3], Vw_d[3], 0)
            half(nc.gpsimd, w_pre[3], Vw_d[3], 1)
            nc.gpsimd.dma_start(out=w_pre[4][:, :], in_=Vw_d[4])
            half(nc.sync, hT_t[0], hT_d[0], 0)
            half(nc.scalar, cb_t[0], cbT_d[0], 0)
            for i in range(1, 8):
                half(nc.sync, hT_t[i], hT_d[i], 0)
                half(nc.scalar, cb_t[i], cbT_d[i], 0)
            # mg1 halves + epilogue srcs, needed from ~130us onward
            for i in range(8):
                half(nc.sync, hT_t[i], hT_d[i], 1)
            xc_t = []
            for xt in range(6):
                t_ = srcp.tile([128, BC], BF, tag=f"xc{xt}", name=f"xc{xt}")
                nc.scalar.dma_start(out=t_[:, :], in_=xc_d[xt])
                xc_t.append(t_)
            co_t = [None] * 8
            for q in range(8):       # c_tm1 [128,256] per (mg, mloc)
                t_ = srcp.tile([128, 256], F32, tag=f"co{q}", name=f"co{q}")
                nc.sync.dma_start(out=t_[:, :],
                                  in_=cold_d[q * 128:(q + 1) * 128, :])
                co_t[q] = t_
            for i in range(8):
                half(nc.scalar, cb_t[i], cbT_d[i], 1)

            warm_src = nc.alloc_sbuf_tensor("warm_src", [128, 512], BF).ap()
            warm = psp.tile([128, 512], F32, tag="q0", name="warm")
            for _ in range(16):
                nc.tensor.matmul(warm[:, :], warm_src[:, 0:128],
                                 warm_src[:, 0:512], start=True, stop=True)

            if with_bias:
                ones_t = srcp.tile([128, 512], BF, tag="ones", name="ones")
                nc.vector.memset(ones_t[:, :], 1.0)

            def pw_head(bank, mloc, mg):
                # phase 1: everything that only needs p0 (i|f) + c_tm1
                p0 = bank[mloc]
                co = co_t[mg * 4 + mloc]
                if_s = pwp.tile([128, 512], F32, tag="if", name="ift")
                nc.scalar.activation(if_s[:, :], p0[:, :], AF.Sigmoid)
                t2 = pwp.tile([128, 256], F32, tag="t2", name="t2t")
                nc.vector.tensor_mul(t2[:, :], if_s[:, 256:512], co[:, :])
                return if_s, t2

            def pw_tail(bank, mloc, mg, if_s, t2, final=False):
                # phase 2: needs p1 (g|o); c_out is DMA'd before the h
                # chain so the last store is h only.  The kernel-final
                # variant ships the h store as two halves on two queues.
                row0 = mg * 512 + mloc * 128
                p1 = bank[4 + mloc]
                g_s = pwp.tile([128, 256], F32, tag="g", name="gt")
                nc.scalar.activation(g_s[:, :], p1[:, 0:256], AF.Tanh)
                t1 = pwp.tile([128, 256], F32, tag="t1", name="t1t")
                nc.vector.tensor_mul(t1[:, :], if_s[:, 0:256], g_s[:, :])
                c_n = iop.tile([128, 256], F32, tag="cn", name="cnt")
                nc.vector.tensor_add(c_n[:, :], t1[:, :], t2[:, :])
                nc.sync.dma_start(out=c_out_d[row0:row0 + 128, :],
                                  in_=c_n[:, :])
                o_s = pwp.tile([128, 256], F32, tag="o", name="ot")
                nc.scalar.activation(o_s[:, :], p1[:, 256:512], AF.Sigmoid)
                th = pwp.tile([128, 256], F32, tag="th", name="tht")
                nc.scalar.activation(th[:, :], c_n[:, :], AF.Tanh)
                h_n = iop.tile([128, 256], F32, tag="hn", name="hnt")
                nc.vector.tensor_mul(h_n[:, :], o_s[:, :], th[:, :])
                if final:
                    nc.sync.dma_start(out=h_out_d[row0:row0 + 128, 0:128],
                                      in_=h_n[:, 0:128])
                    nc.scalar.dma_start(out=h_out_d[row0:row0 + 128, 128:256],
                                        in_=h_n[:, 128:256])
                else:
                    nc.sync.dma_start(out=h_out_d[row0:row0 + 128, :],
                                      in_=h_n[:, :])

            def pointwise(bank, mlocs, mg):
                if len(mlocs) != 3:
                    for mloc in mlocs:
                        if_s, t2 = pw_head(bank, mloc, mg)
                        pw_tail(bank, mloc, mg, if_s, t2)
                    return
                # wave-A fused variant: the three c_n live in one tile so
                # a single [128,768] tanh replaces three [128,256] ones,
                # shortening the scalar queue that gates wave B's chain.
                c_all = iop.tile([128, 768], F32, tag="cna", name="cnat")
                th_all = pwp.tile([128, 768], F32, tag="tha", name="that")
                os_ = {}
                for i, mloc in enumerate(mlocs):
                    row0 = mg * 512 + mloc * 128
                    p0, p1 = bank[mloc], bank[4 + mloc]
                    co = co_t[mg * 4 + mloc]
                    if_s = pwp.tile([128, 512], F32, tag="if", name="ift")
                    nc.scalar.activation(if_s[:, :], p0[:, :], AF.Sigmoid)
                    t2 = pwp.tile([128, 256], F32, tag="t2", name="t2t")
                    nc.vector.tensor_mul(t2[:, :], if_s[:, 256:512],
                                         co[:, :])
                    g_s = pwp.tile([128, 256], F32, tag="g", name="gt")
                    nc.scalar.activation(g_s[:, :], p1[:, 0:256], AF.Tanh)
                    t1 = pwp.tile([128, 256], F32, tag="t1", name="t1t")
                    nc.vector.tensor_mul(t1[:, :], if_s[:, 0:256],
                                         g_s[:, :])
                    cs = slice(i * 256, (i + 1) * 256)
                    nc.vector.tensor_add(c_all[:, cs], t1[:, :], t2[:, :])
                    nc.sync.dma_start(out=c_out_d[row0:row0 + 128, :],
                                      in_=c_all[:, cs])
                    o_s = pwp.tile([128, 256], F32, tag="o", name="ot")
                    nc.scalar.activation(o_s[:, :], p1[:, 256:512],
                                         AF.Sigmoid)
                    os_[i] = o_s
                nc.scalar.activation(th_all[:, :], c_all[:, :], AF.Tanh)
                for i, mloc in enumerate(mlocs):
                    row0 = mg * 512 + mloc * 128
                    cs = slice(i * 256, (i + 1) * 256)
                    h_n = iop.tile([128, 256], F32, tag="hn", name="hnt")
                    nc.vector.tensor_mul(h_n[:, :], os_[i][:, :],
                                         th_all[:, cs])
                    nc.sync.dma_start(out=h_out_d[row0:row0 + 128, :],
                                      in_=h_n[:, :])

            for mg in range(2):           # groups of 4 batch tiles (512 rows)
                bsl = slice(mg * 512, (mg + 1) * 512)
                bank = [psp.tile([128, 512], F32, tag=f"q{q}", name=f"q{q}")
                        for q in range(8)]
                w_tiles = {}
                hc_tiles = {}

                def mm(t, mlocs, nhs=(0, 1), bank=bank, w_tiles=w_tiles,
                       hc_tiles=hc_tiles):
                    hc, w_t = hc_tiles[t], w_tiles[t]
                    for mloc in mlocs:
                        for nh in nhs:
                            nc.tensor.matmul(
                                bank[nh * 4 + mloc][:, :],
                                hc[:, mloc * 128:(mloc + 1) * 128],
                                w_t[:, nh * 512:(nh + 1) * 512],
                                start=(t == 0), stop=(t == ntt - 1))

                def mm_chunked(t, csize, bank=bank, w_tiles=w_tiles,
                               hc_tiles=hc_tiles):
                    # chunk-major emission: the first matmuls need only
                    # the first csize weight columns, so the stream can
                    # start before the whole tile has landed.
                    # start=True clears has_written for the WHOLE bank,
                    # so only the first chunk touching a bank may carry
                    # it; later chunks land on cleared bits and the
                    # accumulate-mode write overwrites there anyway.
                    hc, w_t = hc_tiles[t], w_tiles[t]
                    for c0 in range(0, NC, csize):
                        nh, p0 = c0 // 512, c0 % 512
                        for mloc in range(4):
                            nc.tensor.matmul(
                                bank[nh * 4 + mloc][:, p0:p0 + csize],
                                hc[:, mloc * 128:(mloc + 1) * 128],
                                w_t[:, c0:c0 + csize],
                                start=(t == 0 and p0 == 0),
                                stop=(t == ntt - 1))

                # mg0 runs all 8 banks in lockstep (weight-DMA demand at
                # startup stays at one tile per 8 matmuls); mg1 runs the
                # lagged wave schedule so its wave-A pointwise overlaps
                # the matmul tail (its weights prefetch during mg0).
                lag0 = 0 if mg == 0 else LAG
                lag1 = 0 if mg == 0 else LAG + LAGB
                wb_state = [None]
                for s in range(ntt + lag1):
                    if s < ntt:
                        t = s
                        if mg == 0 and t in w_pre:
                            w_t = w_pre[t]     # prefetched pre-warmup
                        else:
                            w_t = wp.tile([128, NC], BF, tag="w", name="wt")
                            # gpsimd queue: don't serialize behind srcs
                            nc.gpsimd.dma_start(out=w_t[:, :], in_=Vw_d[t])
                        w_tiles[t] = w_t
                        if t < 64:
                            ub, kb = ORDER[t]
                            hc = hcp.tile([128, 512], BF, tag="hc",
                                          name="hct")
                            nc.vector.tensor_mul(
                                hc[:, :], hT_t[ub][:, bsl],
                                cb_t[kb][:, bsl])
                        elif t < 70:
                            hc = xc_t[t - 64][:, bsl]
                        else:
                            hc = ones_t
                        hc_tiles[t] = hc
                        if mg == 0 and t in (1, 2):
                            mm_chunked(t, 256)         # chunk-major
                        elif mg == 0 and t in (0, 3, 4):
                            mm_chunked(t, 512)         # nh-major halves
                        else:
                            mm(t, (0, 1, 2))           # wave A
                            if mg == 0:
                                mm(t, (3,))
                    if mg == 1:
                        if lag0 <= s < ntt + lag0:
                            mm(s - lag0, (3,), nhs=(0,))    # wave B nh=0
                            if s - lag0 == ntt - 1:
                                wb_state[0] = pw_head(bank, 3, mg)
                        if lag1 <= s:
                            tb = s - lag1                   # wave B nh=1
                            if tb == ntt - 1:
                                # g-half first so the final tanh starts
                                # one matmul earlier
                                hcb, w_t = hc_tiles[tb], w_tiles[tb]
                                for p0 in (0, 256):
                                    nc.tensor.matmul(
                                        bank[7][:, p0:p0 + 256],
                                        hcb[:, 384:512],
                                        w_t[:, 512 + p0:768 + p0],
                                        start=False, stop=True)
                            else:
                                mm(tb, (3,), nhs=(1,))
                            del hc_tiles[tb]
                            del w_tiles[tb]
                    if s == ntt - 1:
                        pointwise(bank, (0, 1, 2), mg)   # wave A epilogue
                if mg == 0:
                    pointwise(bank, (3,), mg)
                else:
                    pw_tail(bank, 3, mg, *wb_state[0], final=True)

    _split_excess_waits(nc)
    return nc


def _prep_in_maps(inputs, h_tm1, c_tm1, basis_kernel, basis_recurrent_kernel,
                  bias, with_bias):
    inputs = np.asarray(inputs, np.float32)
    x = inputs[:, :E]
    c_prob = inputs[:, E:]
    h_tm1 = np.asarray(h_tm1, np.float32)
    c_tm1 = np.asarray(c_tm1, np.float32)

    Vr = np.asarray(basis_recurrent_kernel, np.float32) \
        .transpose(1, 0, 2).reshape(K, 8, 128, 4 * U)
    # permute the 64 recurrent tiles into the device's ORDER
    Vr = np.concatenate([Vr[kb, ub] for ub, kb in ORDER], 0)
    Wx = np.asarray(basis_kernel, np.float32) \
        .transpose(1, 0, 2).reshape(K * E, 4 * U)
    parts = [Vr, Wx]
    if with_bias:
        Bt = np.zeros((128, 4 * U), np.float32)
        Bt[0] = np.asarray(bias, np.float32)
        parts.append(Bt)
    Vw_full = np.concatenate(parts, 0)           # [ntt*128, 4096]
    ntt = Vw_full.shape[0] // 128

    er = np.arange(K * E) % E                    # x-part row -> e
    kr = np.arange(K * E) // E                   # x-part row -> k

    in_maps = []
    for g in range(G):
        bsl = slice(g * BC, (g + 1) * BC)
        hT = np.ascontiguousarray(h_tm1[bsl].T).astype(bfloat16) \
            .reshape(8, 128, BC)
        # x-part moving operand premultiplied on the host (one fp32
        # multiply + single bf16 round, slightly better than the
        # device's bf16*bf16)
        xc = np.ascontiguousarray(x[bsl].T[er] * c_prob[bsl].T[kr]) \
            .astype(bfloat16).reshape(6, 128, BC)
        cbT = np.ascontiguousarray(
            np.broadcast_to(c_prob[bsl].T[:, None, :], (8, 128, BC))
        ).astype(bfloat16)
        for j in range(J):
            cols = np.concatenate(
                [np.arange(gt * U + j * 256, gt * U + (j + 1) * 256)
                 for gt in range(4)])
            Vw_c = np.ascontiguousarray(Vw_full[:, cols]).astype(bfloat16) \
                .reshape(ntt, 128, NC)
            co = np.ascontiguousarray(c_tm1[bsl, j * 256:(j + 1) * 256])
            in_maps.append({"hT": hT, "xc": xc, "cbT": cbT,
                            "Vw": Vw_c, "c_tm1": co})
    return in_maps


def _run(in_maps, with_bias, trace=False, **kw):
    key = f"nc{int(with_bias)}"
    if key not in _cache:
        _cache[key] = _build(with_bias)
    try:
        return run_bass_kernel_spmd(_cache[key], in_maps,
                                    list(range(G * J)), trace=trace, **kw)
    except Exception:
        # transient device-state failures have been observed to clear on
        # the next execution; retry once without tracing
        return run_bass_kernel_spmd(_cache[key], in_maps,
                                    list(range(G * J)), trace=False)


def kernel(inputs, h_tm1, c_tm1, basis_kernel, basis_recurrent_kernel, bias,
           _trace=False, **_kw):
    with_bias = bool(np.any(np.asarray(bias)))
    in_maps = _prep_in_maps(inputs, h_tm1, c_tm1, basis_kernel,
                            basis_recurrent_kernel, bias, with_bias)
    res = _run(in_maps, with_bias, trace=_trace, **_kw)
    h = np.empty((B, U), np.float32)
    c = np.empty((B, U), np.float32)
    for g in range(G):
        for j in range(J):
            r = res.results[g * J + j]
            h[g * BC:(g + 1) * BC, j * 256:(j + 1) * 256] = r["h_out"]
            c[g * BC:(g + 1) * BC, j * 256:(j + 1) * 256] = r["c_out"]
    kernel.last_results = res
    return (h, c)
